# revision 3
# baseline (speedup 1.0000x reference)
"""BinaryLinear (4,2048,4096)x(4096,4096) on 8 TRN2 NeuronCores.

y = x @ (scale * sign(w)).T with scale = mean(|w|, axis=1).

Strategy: data-parallel over the 8192 flattened rows of x (1024 rows/core),
weight replicated. Per core:
  - x^T shard is DMA'd with an on-the-fly cast to bf16 and cached in SBUF.
  - w^T streams through SBUF in [128k x 512n] fp32 tiles; ACT computes
    sign(w)->bf16 tiles (exact +-1), DVE computes |w| and accumulates the
    per-column sums in fp32; a single fp32 matmul with a (1/4096)-constant
    stationary operand reduces the partition dim, broadcasting mean(|w|)
    to every PSUM partition.
  - Main compute: 2048 bf16 matmuls (lhsT = x^T k,m-tile, rhs = sign tile)
    accumulating over k into PSUM; the PSUM->SBUF copy is fused with the
    per-column scale multiply on DVE; fp32 results DMA out.
The sign matrix is exact in bf16, so the only precision loss vs the fp32
reference is the bf16 rounding of x (~1e-3 relative).
"""

import sys

for _p in ("/opt/trn_rl_repo",):
    if _p not in sys.path:
        sys.path.append(_p)

import numpy as np

import concourse.bass as bass
import concourse.mybir as mybir
import concourse.tile as tile
from concourse import bacc
from concourse.bass_utils import run_bass_kernel_spmd

P = 128
K_DIM = 4096          # contraction (in_chn)
KT = K_DIM // P       # 32 k-tiles
N_DIM = 4096          # out_chn
NT = 512              # n tile (PSUM bank width in fp32)
N_TILES = N_DIM // NT
N_CORES = 8
M_FULL = 4 * 2048     # flattened batch rows
M_LOC = M_FULL // N_CORES
MT = M_LOC // P

f32 = mybir.dt.float32
bf16 = mybir.dt.bfloat16


def build_kernel(repeat: int = 1):
    nc = bacc.Bacc("TRN2", target_bir_lowering=False)
    xt = nc.dram_tensor("xt", [K_DIM, M_LOC], f32, kind="ExternalInput")
    wt = nc.dram_tensor("wt", [K_DIM, N_DIM], f32, kind="ExternalInput")
    y = nc.dram_tensor("y", [M_LOC, N_DIM], f32, kind="ExternalOutput")

    xt_r = xt.rearrange("(kt p) m -> p kt m", p=P)
    wt_r = wt.rearrange("(kt p) n -> p kt n", p=P)
    y_r = y.rearrange("(mt p) n -> p mt n", p=P)

    with tile.TileContext(nc) as tc:
        with (
            tc.tile_pool(name="xcache", bufs=1) as xcache_pool,
            tc.tile_pool(name="const", bufs=1) as const_pool,
            tc.tile_pool(name="wstage", bufs=3) as wstage_pool,
            tc.tile_pool(name="absw", bufs=3) as absw_pool,
            tc.tile_pool(name="sgn", bufs=2) as sgn_pool,
            tc.tile_pool(name="acc", bufs=2) as acc_pool,
            tc.tile_pool(name="scale", bufs=2) as scale_pool,
            tc.tile_pool(name="out", bufs=4) as out_pool,
            tc.tile_pool(name="psum_s", bufs=2, space="PSUM") as psum_s_pool,
            tc.tile_pool(name="psum_y", bufs=4, space="PSUM") as psum_y_pool,
        ):
            ones = const_pool.tile([P, P], f32, tag="ones")
            nc.vector.memset(ones[:], 1.0 / K_DIM)
            xcache = xcache_pool.tile([P, KT, M_LOC], bf16, tag="xc")

            def body(_i=None):
                # Load + cast x^T shard to bf16 (SWDGE casts in-flight).
                for c in range(0, KT, 4):
                    nc.gpsimd.dma_start(
                        xcache[:, c : c + 4, :], xt_r[:, c : c + 4, :]
                    )

                for nt_i in range(N_TILES):
                    nsl = bass.ts(nt_i, NT)
                    sgn = sgn_pool.tile([P, KT, NT], bf16, tag="sgn")
                    acc = acc_pool.tile([P, NT], f32, tag="acc")
                    for kc in range(0, KT, 4):
                        wstage = wstage_pool.tile([P, 4, NT], f32, tag="ws")
                        nc.sync.dma_start(
                            wstage[:], wt_r[:, kc : kc + 4, nsl]
                        )
                        for j in range(4):
                            k = kc + j
                            nc.scalar.sign(sgn[:, k, :], wstage[:, j, :])
                            # |w| exactly via sign-bit clear on DVE
                            # (abs_max is not a valid TRN2 tensor op).
                            if k == 0:
                                abs_dst = acc[:]
                            else:
                                absw = absw_pool.tile(
                                    [P, NT], f32, tag="absw", name="absw"
                                )
                                abs_dst = absw[:]
                            nc.vector.tensor_scalar(
                                abs_dst.bitcast(mybir.dt.uint32),
                                wstage[:, j, :].bitcast(mybir.dt.uint32),
                                0x7FFFFFFF, None,
                                mybir.AluOpType.bitwise_and,
                            )
                            if k > 0:
                                nc.vector.tensor_tensor(
                                    acc[:], acc[:], abs_dst,
                                    mybir.AluOpType.add,
                                )
                    # Reduce acc over partitions (fp32 matmul with 1/K ones);
                    # every PSUM partition receives the same column sums, so
                    # the result is mean(|w|) broadcast over partitions.
                    scale_ps = psum_s_pool.tile([P, NT], f32, tag="scale_ps")
                    nc.tensor.matmul(
                        scale_ps[:], lhsT=ones[:], rhs=acc[:],
                        start=True, stop=True,
                    )
                    scale_sb = scale_pool.tile([P, NT], f32, tag="scale_sb")
                    nc.vector.tensor_copy(scale_sb[:], scale_ps[:])

                    for mt_i in range(MT):
                        y_ps = psum_y_pool.tile([P, NT], f32, tag="y_ps")
                        for k in range(KT):
                            nc.tensor.matmul(
                                y_ps[:],
                                lhsT=xcache[:, k, bass.ts(mt_i, P)],
                                rhs=sgn[:, k, :],
                                start=(k == 0),
                                stop=(k == KT - 1),
                            )
                        out_sb = out_pool.tile([P, NT], f32, tag="out_sb")
                        nc.vector.tensor_tensor(
                            out_sb[:], y_ps[:], scale_sb[:],
                            mybir.AluOpType.mult,
                        )
                        nc.sync.dma_start(y_r[:, mt_i, nsl], out_sb[:])

            if repeat == 1:
                body()
            else:
                with tc.For_i(0, repeat, 1) as _i:
                    body(_i)

    nc.compile()
    return nc


def _shard_inputs(x: np.ndarray, weight: np.ndarray):
    xt = np.ascontiguousarray(
        x.reshape(M_FULL, K_DIM).T
    )  # [K, M_FULL]
    wt = np.ascontiguousarray(weight.T)  # [K, N]
    in_maps = []
    for c in range(N_CORES):
        xt_shard = np.ascontiguousarray(xt[:, c * M_LOC : (c + 1) * M_LOC])
        in_maps.append({"xt": xt_shard, "wt": wt})
    return in_maps


def kernel(x: np.ndarray, weight: np.ndarray) -> np.ndarray:
    x = np.asarray(x, dtype=np.float32)
    weight = np.asarray(weight, dtype=np.float32)
    nc = build_kernel(repeat=1)
    in_maps = _shard_inputs(x, weight)
    res = run_bass_kernel_spmd(nc, in_maps, core_ids=list(range(N_CORES)))
    y = np.concatenate([res.results[c]["y"] for c in range(N_CORES)], axis=0)
    return y.reshape(x.shape[0], x.shape[1], N_DIM).astype(np.float32)
